# revision 12
# baseline (speedup 1.0000x reference)
"""ChebNet (K=5) forward on 8 Trainium2 NeuronCores.

Sharding: nodes partitioned across the 8 cores (graph parallel, 6272
nodes/core with N padded 50000->50176).  Each core owns the
destination-side segment-sum for its node range; the K x 128 x 128 weights
are replicated.  Cross-partition edges are handled by AllGather-ing the
fp16-cast feature matrix each propagate and dma_gather-ing the needed
source rows on the consuming core.

Normalization is folded on the host: selector values are 2*norm_w =
-2*dis[src]*w*dis[dst] (deg/dis computed on host), so on device
  Q'_k = 2 * L_hat @ h_k   with  h_1 = x/2 (k=0 stage scale 0.5)
  T_1  = Q'_1,   T_k = Q'_k - T_{k-2}  (k >= 2)

Per propagate, on each core:
  hbuf = AllGather(fp16(T_{k-1}))                (fp16, [50176, 128] Shared DRAM)
  G    = dma_gather(hbuf, src idx)               (fp16, prepare_only + trigger,
                                                  4 SWDGE queues round-robin)
  Q'   = sum_chunks S_chunk.T @ G_chunk          (TensorE, f32 PSUM)
  T_k  = Q' - T_{k-2}                            (DVE, reads PSUM directly)
  out += T_k @ W_k                               (PE transpose + fp16 matmul)

dma_gather indices are int16, so gathers are windowed into the two
32768-row halves of hbuf (edges grouped per (dst-block, src-half)).
All 8 cores run one SPMD program: per-(block,half) chunk counts are max'd
over cores; padding slots gather row 0 with weight 0.
"""

import os
import numpy as np

N = 50000
E = 600000
C = 128
K = 5
NC = 8
PB = 128                      # nodes per dst block
SLICE = 6272                  # nodes per core
NPAD = SLICE * NC             # 50176
NBLK = SLICE // PB            # 49
HALF = 32768                  # int16 gather window (rows)
GCALL_MAX = 64                # max 128-idx chunks per dma_gather call
SLAB = 32                     # selector chunks per DMA slab
SGB = 12                      # dst blocks per supergroup
NQ = 4                        # SWDGE queues for gather round-robin

F16 = np.float16
F32 = np.float32


# ----------------------------------------------------------------------
# host-side plan
# ----------------------------------------------------------------------

def build_plan(edge_index, edge_weight):
    src = edge_index[0].astype(np.int64)
    dst = edge_index[1].astype(np.int64)
    w = edge_weight.astype(F32)

    # host-folded symmetric normalization (matches reference):
    # deg = segment_sum(w, src); dis = where(deg>0, rsqrt(max(deg,1e-20)), 0)
    deg = np.bincount(src, weights=w.astype(np.float64), minlength=N).astype(F32)
    dis = np.where(deg > 0, 1.0 / np.sqrt(np.maximum(deg, F32(1e-20))), 0.0)
    dis = dis.astype(F32)
    sval = (-2.0 * dis[src] * w * dis[dst]).astype(F32)

    # ---- per-core block-balancing permutation ----
    # Node->(block,slot) assignment within a core is free (host un-permutes
    # the output), so bin-pack nodes per core to keep per-(block,half) edge
    # counts <= 6*128, minimizing chunk padding. Halves for balancing are
    # estimated with the identity layout (only core-5 sources straddle HALF).
    half0 = ((src // SLICE) * SLICE + (src % PB) * NBLK
             + (src % SLICE) // PB >= HALF).astype(np.int64)
    CAP = 6 * PB
    perm_all = np.zeros((NC, SLICE), np.int64)    # old local -> new local
    for c in range(NC):
        base = c * SLICE
        sel = (dst >= base) & (dst < base + SLICE)
        dl = dst[sel] - base
        hh = half0[sel]
        c0 = np.bincount(dl[hh == 0], minlength=SLICE)
        c1 = np.bincount(dl[hh == 1], minlength=SLICE)
        order_n = np.argsort(-(c0 + c1), kind="stable")
        s0 = np.zeros(NBLK, np.int64)
        s1 = np.zeros(NBLK, np.int64)
        cnt = np.zeros(NBLK, np.int64)
        newpos = np.zeros(SLICE, np.int64)
        spill = []
        for n in order_n:
            ok = np.nonzero((cnt < PB) & (s0 + c0[n] <= CAP)
                            & (s1 + c1[n] <= CAP))[0]
            if len(ok):
                slack = np.minimum(CAP - s0[ok] - c0[n], CAP - s1[ok] - c1[n])
                b = ok[np.argmax(slack)]
                newpos[n] = b * PB + cnt[b]
                cnt[b] += 1
                s0[b] += c0[n]
                s1[b] += c1[n]
            else:
                spill.append(n)
        for i, n in enumerate(spill):
            ok = np.nonzero(cnt < PB)[0]
            b = ok[-1]                            # concentrate overflow high
            newpos[n] = b * PB + cnt[b]
            cnt[b] += 1
        perm_all[c] = newpos

    src_new = (src // SLICE) * SLICE + perm_all[src // SLICE, src % SLICE]
    dst_new = (dst // SLICE) * SLICE + perm_all[dst // SLICE, dst % SLICE]

    # ---- dst-side (propagate) ----
    # virtual row index for the p-major AllGather layout:
    # node n = r*SLICE + t*PB + p  ->  m = r*SLICE + p*NBLK + t
    mr = src_new // SLICE
    mt = (src_new % SLICE) // PB
    mp = src_new % PB
    msrc = mr * SLICE + mp * NBLK + mt
    dst = dst_new
    core = dst // SLICE
    blk = (dst % SLICE) // PB
    half = (msrc >= HALF).astype(np.int64)
    key = (core * NBLK + blk) * 2 + half
    order = np.argsort(key, kind="stable")
    counts = np.bincount(key, minlength=NC * NBLK * 2).reshape(NC, NBLK, 2)
    bounds = np.concatenate([[0], np.cumsum(counts.reshape(-1))])
    NCH = (-(-counts // PB)).max(axis=0)          # [NBLK, 2] uniform

    sg_list = [list(range(b, min(b + SGB, NBLK))) for b in range(0, NBLK, SGB)]

    chunk_seq = []                                # (b, h) per chunk
    segments = []                                 # (h, [chunk ids]) per (sg,h)
    for sg in sg_list:
        for h in (0, 1):
            seg = []
            for b in sg:
                for _ in range(NCH[b][h]):
                    seg.append(len(chunk_seq))
                    chunk_seq.append((b, h))
            if seg:
                segments.append((h, seg))
    TOTCH = len(chunk_seq)

    gcalls = []                                   # (h, chunk0, nch, col0)
    idx_cols = 0
    for h, seg in segments:
        i = 0
        while i < len(seg):
            n = min(GCALL_MAX, len(seg) - i)
            gcalls.append((h, seg[i], n, idx_cols))
            idx_cols += n * PB // 16
            i += n

    nslab = -(-TOTCH // SLAB)
    idx_all = np.zeros((NC, 128, idx_cols), np.int16)
    sv_all = np.zeros((NC, nslab, PB, SLAB * PB), F16)
    for c in range(NC):
        per_src = np.zeros((TOTCH, PB), np.int64)
        per_w = np.zeros((TOTCH, PB), F32)
        per_dl = np.zeros((TOTCH, PB), np.int64)
        ch_of = {}
        for ci, (b, h) in enumerate(chunk_seq):
            ch_of.setdefault((b, h), []).append(ci)
        for b in range(NBLK):
            for h in (0, 1):
                kk = (c * NBLK + b) * 2 + h
                eids = order[bounds[kk]:bounds[kk + 1]]
                for j, ci in enumerate(ch_of.get((b, h), [])):
                    sl = eids[j * PB:(j + 1) * PB]
                    n = len(sl)
                    per_src[ci, :n] = msrc[sl] - h * HALF
                    per_w[ci, :n] = sval[sl]
                    per_dl[ci, :n] = dst[sl] % PB
        for h, ci0, n, col0 in gcalls:
            flat = per_src[ci0:ci0 + n].reshape(-1)
            colb = flat.reshape(-1, 16).T.astype(np.int16)   # [16, n*8]
            idx_all[c, :, col0:col0 + n * 8] = np.tile(colb, (8, 1))
        ar = np.arange(PB)
        for ci in range(TOTCH):
            s, o = ci // SLAB, ci % SLAB
            t = np.zeros((PB, PB), F32)
            t[ar, per_dl[ci]] = per_w[ci]
            sv_all[c, s, :, o * PB:(o + 1) * PB] = t.astype(F16)

    sched = dict(NCH=NCH, chunk_seq=chunk_seq, segments=segments,
                 gcalls=gcalls, idx_cols=idx_cols, TOTCH=TOTCH,
                 nslab=nslab, sg_list=sg_list)
    return sched, idx_all, sv_all, perm_all


# ----------------------------------------------------------------------
# device program
# ----------------------------------------------------------------------

def build_program(sched, no_collective=False, no_gather=False, repeat=1, abl=()):
    import concourse.bass as bass
    import concourse.bacc as bacc
    import concourse.mybir as mybir
    import concourse.tile as tile

    dt = mybir.dt
    AF = mybir.ActivationFunctionType
    ALU = mybir.AluOpType
    RG = [list(range(NC))]

    NCH = sched["NCH"]
    chunk_seq = sched["chunk_seq"]
    gcalls = sched["gcalls"]
    idx_cols = sched["idx_cols"]
    nslab = sched["nslab"]

    nc = bacc.Bacc("TRN2", target_bir_lowering=False, debug=False,
                   num_devices=NC, num_swdge_queues=NQ)

    x_h = nc.dram_tensor("x_pm", [128, NBLK * C], dt.float32, kind="ExternalInput")
    idx_h = nc.dram_tensor("idx", [128, idx_cols], dt.int16, kind="ExternalInput")
    sv_h = nc.dram_tensor("sv", [nslab, PB, SLAB * PB], dt.float16, kind="ExternalInput")
    w_h = nc.dram_tensor("wmat", [C, K * C], dt.float16, kind="ExternalInput")
    bias_h = nc.dram_tensor("biasb", [128, C], dt.float32, kind="ExternalInput")
    id_h = nc.dram_tensor("ident", [128, 128], dt.float32, kind="ExternalInput")
    out_h = nc.dram_tensor("out_sl", [SLICE, C], dt.float32, kind="ExternalOutput")

    chunk_call = {}
    for gi, (h, ci0, n, col0) in enumerate(gcalls):
        for j in range(n):
            chunk_call[ci0 + j] = (gi, j)

    with tile.TileContext(nc) as tc:
        import contextlib
        ctx = contextlib.ExitStack()
        with ctx:
            const = ctx.enter_context(tc.tile_pool(name="const", bufs=1))
            big = ctx.enter_context(tc.tile_pool(name="big", bufs=1))
            dram = ctx.enter_context(tc.tile_pool(name="dram", bufs=2, space="DRAM"))

            # resident tiles
            idx_sb = const.tile([128, idx_cols], dt.int16)
            nc.sync.dma_start(idx_sb[:], idx_h[:, :])
            w_sb = const.tile([C, K * C], dt.float16)
            nc.sync.dma_start(w_sb[:], w_h[:, :])
            bias_sb = const.tile([128, C], dt.float32)
            nc.sync.dma_start(bias_sb[:], bias_h[:, :])
            id_sb = const.tile([128, 128], dt.float32)
            nc.sync.dma_start(id_sb[:], id_h[:, :])

            # big state: T slots, out_acc  ([128, NBLK*C] f32)
            slots = [big.tile([128, NBLK * C], dt.float32, tag=f"slot{i}",
                              name=f"slot{i}") for i in range(3)]
            oacc = big.tile([128, NBLK * C], dt.float32)

            # main-loop pools
            gp = ctx.enter_context(tc.tile_pool(name="gp", bufs=2))
            sp = ctx.enter_context(tc.tile_pool(name="sp", bufs=3))
            hstg = ctx.enter_context(tc.tile_pool(name="hstg", bufs=2))
            tstg = ctx.enter_context(tc.tile_pool(name="tstg", bufs=3))
            ostg = ctx.enter_context(tc.tile_pool(name="ostg", bufs=3))
            qps = ctx.enter_context(tc.tile_pool(name="qps", bufs=4, space="PSUM"))
            tps = ctx.enter_context(tc.tile_pool(name="tps", bufs=2, space="PSUM"))
            ops = ctx.enter_context(tc.tile_pool(name="ops", bufs=2, space="PSUM"))

            def band(t, b):
                return t[:, b * C:(b + 1) * C]

            def hstage_and_ag(tsrc, scale):
                """ag_in = fp16(scale * tsrc) in p-major; AllGather into hbuf.
                hbuf rows are virtual: m = r*SLICE + p*NBLK + t."""
                hst = hstg.tile([128, NBLK * C], dt.float16, tag="hs", name="hs")
                nc.vector.tensor_scalar(hst[:, :], tsrc[:, :], scale, None,
                                        ALU.mult)
                ag_in = dram.tile([128, NBLK * C], dt.float16, tag="agin",
                                  name="ag_in")
                nc.sync.dma_start(ag_in[:, :], hst[:, :])
                hbuf = dram.tile([NC * 128, NBLK * C], dt.float16, tag="hbuf",
                                 name="hbuf")
                if no_collective:
                    nc.sync.dma_start(hbuf[:128, :], ag_in[:, :])
                else:
                    nc.gpsimd.collective_compute(
                        "AllGather", mybir.AluOpType.bypass, replica_groups=RG,
                        ins=[ag_in.opt()], outs=[hbuf.opt()])
                return hbuf.rearrange("a (t c) -> (a t) c", c=C)

            def out_phase(tsrc, k, first):
                for b in range(NBLK):
                    tp = tps.tile([128, 128], dt.float32, tag="tp", name="tp")
                    nc.tensor.transpose(tp[:], band(tsrc, b), id_sb[:])
                    tt = tstg.tile([128, 128], dt.float16, tag="tt", name="tt")
                    nc.vector.tensor_copy(tt[:], tp[:])
                    op = ops.tile([128, C], dt.float32, tag="op", name="op")
                    nc.tensor.matmul(op[:], tt[:], w_sb[:, k * C:(k + 1) * C],
                                     start=True, stop=True)
                    if first:
                        nc.scalar.copy(band(oacc, b), op[:])
                    else:
                        nc.vector.tensor_tensor(band(oacc, b), band(oacc, b),
                                                op[:], ALU.add)

            def propagate(hbuf, k, dst_t, prev2_t):
                """gathers + S-matmuls; dst_t band = Q' [- prev2_t band]."""
                sv_tiles = {}
                q_open = {}
                g_tiles = [None] * len(gcalls)
                done = {}
                first_done = {}
                for ci, (b, h) in enumerate(chunk_seq):
                    gi, j = chunk_call[ci]
                    if g_tiles[gi] is None:
                        hh, cc0, n, col0 = gcalls[gi]
                        gt = gp.tile([128, GCALL_MAX, C], dt.float16,
                                     tag="g", name="gt")
                        if no_gather:
                            nc.sync.dma_start(
                                gt[:, :n, :],
                                hbuf[:n * 128, :].rearrange(
                                    "(n p) c -> p n c", p=128))
                        else:
                            src_ap = (hbuf[:HALF, :] if hh == 0
                                      else hbuf[HALF:NPAD, :])
                            nc.gpsimd.dma_gather(
                                gt[:, :n, :], src_ap,
                                idx_sb[:, col0:col0 + n * 8],
                                num_idxs=n * 128, num_idxs_reg=n * 128,
                                elem_size=C, single_packet=False)
                        g_tiles[gi] = gt
                    gt = g_tiles[gi]
                    s, o = ci // SLAB, ci % SLAB
                    if s not in sv_tiles:
                        st = sp.tile([128, SLAB * PB], dt.float16,
                                     tag="s", name="svt")
                        nc.sync.dma_start(st[:], sv_h[s, :, :])
                        sv_tiles = {s: st}
                    st = sv_tiles[s]
                    nd = done.get((b, h), 0)
                    if (b, h) not in q_open:
                        q_open[(b, h)] = qps.tile([128, C], dt.float32,
                                                  tag="q", name="q")
                    ps = q_open[(b, h)]
                    last = nd + 1 == NCH[b][h]
                    nc.tensor.matmul(ps[:], st[:, o * PB:(o + 1) * PB],
                                     gt[:, j, :], start=(nd == 0), stop=last)
                    done[(b, h)] = nd + 1
                    if last:
                        if not first_done.get(b):
                            # first closing half of block b
                            if k == 1:
                                nc.vector.tensor_copy(band(dst_t, b), ps[:])
                            else:
                                nc.vector.tensor_tensor(band(dst_t, b), ps[:],
                                                        band(prev2_t, b),
                                                        ALU.subtract)
                            first_done[b] = True
                        else:
                            nc.vector.tensor_tensor(band(dst_t, b),
                                                    band(dst_t, b), ps[:],
                                                    ALU.add)
                        del q_open[(b, h)]
                for b in range(NBLK):
                    if NCH[b][0] == 0 and NCH[b][1] == 0:
                        if k == 1:
                            nc.vector.memset(band(dst_t, b), 0.0)
                        else:
                            nc.vector.tensor_scalar(band(dst_t, b),
                                                    band(prev2_t, b), -1.0,
                                                    None, ALU.mult)

            def emit_body(rep):
                slot = list(slots)
                # T_0 = x  (p-major, one DMA)
                nc.sync.dma_start(slot[1][:, :], x_h[:, :])

                # k = 0: stage x/2 (selectors carry 2*norm_w)
                hbuf = hstage_and_ag(slot[1], 0.5)
                out_phase(slot[1], 0, True)

                for k in range(1, K):
                    propagate(hbuf, k, slot[2], slot[0])
                    if k < K - 1:
                        hbuf = hstage_and_ag(slot[2], 1.0)
                    out_phase(slot[2], k, False)
                    slot = [slot[1], slot[2], slot[0]]

                # final: out = relu(oacc + bias)
                for b in range(NBLK):
                    fs = ostg.tile([128, C], dt.float32, tag="fs", name="fs")
                    nc.vector.tensor_tensor(fs[:], band(oacc, b), bias_sb[:],
                                            ALU.add)
                    nc.scalar.activation(fs[:], fs[:], AF.Relu)
                    nc.sync.dma_start(out_h[b * PB:(b + 1) * PB, :], fs[:])

            for rep in range(repeat):
                emit_body(rep)

    nc.compile()
    return nc


# ----------------------------------------------------------------------
# entry point
# ----------------------------------------------------------------------

def make_in_maps(inputs, idx_all, sv_all, perm_all):
    x = np.asarray(inputs["x"], F32)
    lw = np.asarray(inputs["lins_w"], F32)
    bias = np.asarray(inputs["bias"], F32)
    xp = np.zeros((NPAD, C), F32)
    xp[:N] = x
    wmat = np.concatenate([lw[k] for k in range(K)], axis=1).astype(F16)
    biasb = np.tile(bias[None, :], (128, 1)).astype(F32)
    ident = np.eye(128, dtype=F32)
    in_maps = []
    for c in range(NC):
        xs = xp[c * SLICE:(c + 1) * SLICE]
        xs_p = np.empty_like(xs)
        xs_p[perm_all[c]] = xs
        x_pm = np.ascontiguousarray(
            xs_p.reshape(NBLK, PB, C).transpose(1, 0, 2).reshape(128, NBLK * C))
        in_maps.append({
            "x_pm": x_pm,
            "idx": np.ascontiguousarray(idx_all[c]),
            "sv": np.ascontiguousarray(sv_all[c]),
            "wmat": wmat,
            "biasb": biasb,
            "ident": ident,
        })
    return in_maps


def _run(inputs, trace=False):
    from concourse import bass_utils

    ei = np.asarray(inputs["edge_index"])
    ew = np.asarray(inputs["edge_weight"], F32)
    sched, idx_all, sv_all, perm_all = build_plan(ei, ew)
    nc = build_program(sched)
    in_maps = make_in_maps(inputs, idx_all, sv_all, perm_all)

    res = bass_utils.run_bass_kernel_spmd(
        nc, in_maps, core_ids=list(range(NC)), trace=trace)
    out = np.concatenate([res.results[c]["out_sl"][perm_all[c]]
                          for c in range(NC)], axis=0)
    return out[:N].astype(F32), res


def kernel(**inputs):
    out, _ = _run(inputs, trace=False)
    return out
